# revision 1
# baseline (speedup 1.0000x reference)
"""Trainium2 Bass kernel for the contrastive loss:

    epos = exp(cos_sim(q_pos, img_pos))   # [2B] rows, D=1024
    eneg = exp(cos_sim(q_neg, img_neg))   # [23B]
    pos_sum = segsum(epos, 2); neg_sum = segsum(eneg, 23)   # [B]
    loss = sum((neg_sum - pos_sum) / (pos_sum + neg_sum + 0.001))

Data-parallel over 8 NeuronCores: core c takes batch items [c*512, (c+1)*512),
i.e. rows [c*1024,(c+1)*1024) of the pos tensors and [c*11776,(c+1)*11776) of
the neg tensors. Each core emits its 512 per-item values; the host sums.

Per-core layout: local item i = 4*p + s (partition p in [0,128), slot s in
[0,4)), so partition p owns pos rows 8p..8p+7 and neg rows 92p..92p+91 of the
core's shard — each partition's rows are contiguous in DRAM, so every DMA is
128 partitions x (4 rows * 4KiB) contiguous.

Per 128-row slice [128, 1024]: the row-wise dot runs on the vector engine as
one fused scalar_tensor_tensor ((a*1)*b with accum_out), and the two
sum-of-squares run on the scalar engine as Square activations with accum_out.
A fraction of the b-squares is moved to the vector engine to balance the two
engines; both stay below the DMA floor (~100 MiB/core through 16 SDMA
engines).

cos and e=exp(cos) are computed per chunk as stats complete, using
1/sqrt(x) = exp(-0.5*ln(x)) so the scalar engine needs only the
natural_log_exp_and_others table set (square/ln/exp) for the entire kernel —
no ~2.7us ACT table switches in the final tail. The tail is just the two
segmented reductions and the per-item fixup.
"""

import ml_dtypes
import numpy as np

import concourse.bass as bass
import concourse.tile as tile
from concourse import mybir
from concourse.bass_utils import run_bass_kernel_spmd

EPS_COS = 1e-8
EP = 0.001

N_CORES = 8
P = 128            # SBUF partitions
D = 1024           # embedding dim
B_FULL = 4096      # total batch items
ITEMS = B_FULL // N_CORES   # 512 items per core
SLOTS = ITEMS // P          # 4 items per partition
J_POS = SLOTS * 2           # 8 pos rows per partition
J_NEG = SLOTS * 23          # 92 neg rows per partition
G = 8                       # j-slices per DMA chunk (4 MiB per tensor)

F32 = mybir.dt.float32
BF16 = mybir.dt.bfloat16
ALU = mybir.AluOpType
ACTF = mybir.ActivationFunctionType


def _split_multiwait_instructions(nc):
    """The walrus build here rejects >1 sync-wait per instruction; hoist extra
    waits onto single-wait NOPs placed just before the instruction."""
    ctr = 0
    for fn in nc.m.functions:
        for bb in fn.blocks:
            insts = list(bb.instructions)
            if not any(
                i.sync_info is not None and len(i.sync_info.on_wait) > 1
                for i in insts
            ):
                continue
            new_insts = []
            for inst in insts:
                si = inst.sync_info
                if si is not None and len(si.on_wait) > 1:
                    waits = list(si.on_wait)
                    is_drain = type(inst).__name__ == "InstDrain"
                    keep = [] if is_drain else waits[-1:]
                    move = waits if is_drain else waits[:-1]
                    for w in move:
                        ctr += 1
                        new_insts.append(
                            mybir.InstNoOp(
                                name=f"I-wsplit-{ctr}",
                                engine=inst.engine,
                                sync_info=mybir.SyncInfo(on_wait=[w], on_update=[]),
                                text_hint="wsplit",
                            )
                        )
                    si.on_wait = keep
                new_insts.append(inst)
            bb.instructions = new_insts


def build_bass():
    nc = bass.Bass()
    qp = nc.declare_dram_parameter("qp", [P * J_POS, D], BF16, isOutput=False)
    pi = nc.declare_dram_parameter("pi", [P * J_POS, D], BF16, isOutput=False)
    qn = nc.declare_dram_parameter("qn", [P * J_NEG, D], BF16, isOutput=False)
    ni = nc.declare_dram_parameter("ni", [P * J_NEG, D], BF16, isOutput=False)
    out = nc.declare_dram_parameter("out", [P, SLOTS], F32, isOutput=True)

    qp_v = qp[:].rearrange("(p j) d -> p j d", j=J_POS)
    pi_v = pi[:].rearrange("(p j) d -> p j d", j=J_POS)
    qn_v = qn[:].rearrange("(p j) d -> p j d", j=J_NEG)
    ni_v = ni[:].rearrange("(p j) d -> p j d", j=J_NEG)

    with tile.TileContext(nc) as tc:
        with (
            tc.tile_pool(name="io", bufs=2) as io,
            tc.tile_pool(name="st", bufs=1) as st,
        ):
            J_ALL = J_POS + J_NEG   # pos stats in cols [0,8), neg in [8,100)
            dot_all = st.tile([P, J_ALL], F32)
            na2_all = st.tile([P, J_ALL], F32)
            nb2_all = st.tile([P, J_ALL], F32)
            e_all = st.tile([P, J_ALL], F32)
            scr_v = st.tile([P, D], BF16)
            scr_s = st.tile([P, D], BF16)

            # Chunk schedule: the last chunks shrink (...,4,2,1,1) so the
            # serial compute after the final input load is minimal.
            def chunk_sizes(total, shrink_tail):
                if not shrink_tail:
                    assert total % G == 0
                    return [G] * (total // G)
                rem = total - 4
                assert rem % G == 0
                return [G] * (rem // G) + [2, 1, 1]

            chunks = []   # (a_view, b_view, col0, j0, gsz)
            for view_a, view_b, col0, total, shrink in (
                (qp_v, pi_v, 0, J_POS, False),
                (qn_v, ni_v, J_POS, J_NEG, True),
            ):
                j0 = 0
                for gsz in chunk_sizes(total, shrink):
                    chunks.append((view_a, view_b, col0, j0, gsz))
                    j0 += gsz
                assert j0 == total

            prod = st.tile([P, J_ALL], F32)
            cosv = st.tile([P, J_ALL], F32)

            # e[:, lo:hi] = exp(dot * exp(-0.5*ln(max(na2*nb2, eps^2)))).
            # ln/exp share the square table set: no ACT table switches.
            def _emit_e(lo, hi):
                c = slice(lo, hi)
                nc.vector.tensor_tensor(
                    out=prod[:, c], in0=na2_all[:, c], in1=nb2_all[:, c],
                    op=ALU.mult,
                )
                nc.vector.tensor_scalar(
                    out=prod[:, c], in0=prod[:, c], scalar1=EPS_COS * EPS_COS,
                    scalar2=None, op0=ALU.max,
                )
                nc.scalar.activation(out=prod[:, c], in_=prod[:, c], func=ACTF.Ln)
                nc.scalar.activation(
                    out=prod[:, c], in_=prod[:, c], func=ACTF.Exp, scale=-0.5
                )
                nc.vector.tensor_tensor(
                    out=cosv[:, c], in0=dot_all[:, c], in1=prod[:, c],
                    op=ALU.mult,
                )
                nc.scalar.activation(
                    out=e_all[:, c], in_=cosv[:, c], func=ACTF.Exp
                )

            # Streaming phase: only dots + squares, no cross-engine chains.
            # 6/11 of b-squares go to the vector engine: per-slice unit cost
            # is ~1.22us on DVE vs ~1.30us on ACT (ACT pays a 185ns
            # ACTIVATION_READ_ACCUMULATOR per accumulate), and ACT also owns
            # all 100 a-squares, so this split equalizes both engines.
            slice_idx = 0
            for a_v, b_v, col0, j0, gsz in chunks:
                a_t = io.tile([P, G, D], BF16, tag="a")
                b_t = io.tile([P, G, D], BF16, tag="b")
                nc.sync.dma_start(out=a_t[:, :gsz, :], in_=a_v[:, j0 : j0 + gsz, :])
                nc.sync.dma_start(out=b_t[:, :gsz, :], in_=b_v[:, j0 : j0 + gsz, :])
                for g in range(gsz):
                    j = col0 + j0 + g
                    a_sl = a_t[:, g, :]
                    b_sl = b_t[:, g, :]
                    nc.vector.scalar_tensor_tensor(
                        out=scr_v[:], in0=a_sl, scalar=1.0, in1=b_sl,
                        op0=ALU.mult, op1=ALU.mult,
                        accum_out=dot_all[:, j : j + 1],
                    )
                    nc.scalar.activation(
                        out=scr_s[:], in_=a_sl, func=ACTF.Square,
                        accum_out=na2_all[:, j : j + 1],
                    )
                    if (slice_idx % 33) < 32:
                        nc.vector.scalar_tensor_tensor(
                            out=scr_v[:], in0=b_sl, scalar=1.0, in1=b_sl,
                            op0=ALU.mult, op1=ALU.mult,
                            accum_out=nb2_all[:, j : j + 1],
                        )
                    else:
                        nc.scalar.activation(
                            out=scr_s[:], in_=b_sl, func=ACTF.Square,
                            accum_out=nb2_all[:, j : j + 1],
                        )
                    slice_idx += 1

                # Once the first 96 columns' stats are complete, compute
                # their e-values while the last chunks still stream in; the
                # final tail then only covers the last 4 columns.
                if col0 + j0 + gsz == 96:
                    _emit_e(0, 96)

            _emit_e(96, J_ALL)

            pos_sum = st.tile([P, SLOTS], F32)
            neg_sum = st.tile([P, SLOTS], F32)
            nc.vector.tensor_reduce(
                out=pos_sum[:],
                in_=e_all[:, :J_POS].rearrange("p (s t) -> p s t", t=2),
                axis=mybir.AxisListType.X,
                op=ALU.add,
            )
            nc.vector.tensor_reduce(
                out=neg_sum[:],
                in_=e_all[:, J_POS:].rearrange("p (s t) -> p s t", t=23),
                axis=mybir.AxisListType.X,
                op=ALU.add,
            )
            num = st.tile([P, SLOTS], F32)
            den = st.tile([P, SLOTS], F32)
            nc.vector.tensor_tensor(
                out=num[:], in0=neg_sum[:], in1=pos_sum[:], op=ALU.subtract
            )
            nc.vector.scalar_tensor_tensor(
                out=den[:], in0=pos_sum[:], scalar=EP, in1=neg_sum[:],
                op0=ALU.add, op1=ALU.add,
            )
            rden = st.tile([P, SLOTS], F32)
            nc.vector.reciprocal(out=rden[:], in_=den[:])
            per_item = st.tile([P, SLOTS], F32)
            nc.vector.tensor_tensor(
                out=per_item[:], in0=num[:], in1=rden[:], op=ALU.mult
            )
            nc.sync.dma_start(out=out[:], in_=per_item[:])

    _split_multiwait_instructions(nc)
    return nc


_NC_CACHE = None


def _get_nc():
    global _NC_CACHE
    if _NC_CACHE is None:
        _NC_CACHE = build_bass()
    return _NC_CACHE


def prepare_in_maps(question_embeddings_pos, question_embeddings_neg,
                    pos_image_embeddings, neg_image_embeddings):
    qp = np.asarray(question_embeddings_pos, dtype=np.float32).astype(ml_dtypes.bfloat16)
    qn = np.asarray(question_embeddings_neg, dtype=np.float32).astype(ml_dtypes.bfloat16)
    pi = np.asarray(pos_image_embeddings, dtype=np.float32).astype(ml_dtypes.bfloat16)
    ni = np.asarray(neg_image_embeddings, dtype=np.float32).astype(ml_dtypes.bfloat16)

    rp = 2 * ITEMS   # pos rows per core
    rn = 23 * ITEMS  # neg rows per core
    return [
        {
            "qp": np.ascontiguousarray(qp[c * rp : (c + 1) * rp]),
            "pi": np.ascontiguousarray(pi[c * rp : (c + 1) * rp]),
            "qn": np.ascontiguousarray(qn[c * rn : (c + 1) * rn]),
            "ni": np.ascontiguousarray(ni[c * rn : (c + 1) * rn]),
        }
        for c in range(N_CORES)
    ]


def kernel(question_embeddings_pos, question_embeddings_neg,
           pos_image_embeddings, neg_image_embeddings, batch_size=None,
           **_unused):
    in_maps = prepare_in_maps(
        question_embeddings_pos, question_embeddings_neg,
        pos_image_embeddings, neg_image_embeddings,
    )
    res = run_bass_kernel_spmd(_get_nc(), in_maps, list(range(N_CORES)))
    total = np.float64(0.0)
    for c in range(N_CORES):
        total += res.results[c]["out"].sum(dtype=np.float64)
    return np.float32(total)



# revision 2
# speedup vs baseline: 2.0346x; 2.0346x over previous
"""Trainium2 Bass kernel for the contrastive loss:

    epos = exp(cos_sim(q_pos, img_pos))   # [2B] rows, D=1024
    eneg = exp(cos_sim(q_neg, img_neg))   # [23B]
    pos_sum = segsum(epos, 2); neg_sum = segsum(eneg, 23)   # [B]
    loss = sum((neg_sum - pos_sum) / (pos_sum + neg_sum + 0.001))

Data-parallel over 8 NeuronCores: core c takes batch items [c*512, (c+1)*512).
Within a core the 12800 rows (1024 pos + 11776 neg) are grouped into 100
row-blocks of 128. All inputs are cast to fp8-e4m3 on the host and stored
TRANSPOSED: tile[p, c, i, r] = X[row r, d = 256c + 128i + p], so the
TensorEngine contracts d along partitions.

Per row-block the three cosine stats (q.q, q.img, img.img) are computed on
the TensorEngine as diagonal blocks: 4 DoubleRow fp8 matmuls (K=256 each)
accumulate diag-bearing [128,128] PSUM blocks; the DVE extracts each
diagonal with one identity-mask scalar_tensor_tensor + accum (~210 ns).
DoubleRow accumulation groups must not interleave (HW weight hazard), so
the 12 matmuls per row-block run group-major.

fp8 is safe here: the final loss is a sum of ~4096 ratios of 2-/23-row
exp(cos) sums; simulated end-to-end fp8-e4m3 error is ~1e-6 relative
(tolerance 2e-2). This halves HBM traffic vs bf16 (25 MiB/core, ~75 us at
the measured ~350 GB/s/core) and moves all elementwise work to the
otherwise-idle PE (~90 us), beating the DVE/ACT-bound bf16 roofline.

Row permutation on host: row-block j<8 holds pos rows, j=2s+k with
column r = item partition; blocks 8..99 hold neg rows, j=8+23s+k. So the
extracted stat columns line up as e_all[p, 2s+k] / e_all[p, 8+23s+k] and
the tail segsum is two tensor_reduce ops, identical to the bf16 kernel.
"""

import ml_dtypes
import numpy as np

import concourse.bass as bass
import concourse.tile as tile
from concourse import mybir
from concourse.bass_utils import run_bass_kernel_spmd

EPS_COS = 1e-8
EP = 0.001

N_CORES = 8
P = 128            # SBUF partitions
D = 1024           # embedding dim
B_FULL = 4096      # total batch items
ITEMS = B_FULL // N_CORES   # 512 items per core
SLOTS = ITEMS // P          # 4 items per partition
RB_POS = 8                  # pos row-blocks per core (1024 rows)
RB_NEG = 92                 # neg row-blocks per core (11776 rows)
RB = RB_POS + RB_NEG        # 100 row-blocks of 128 rows
G_RB = 4                    # row-blocks per DMA group (512 KiB per tensor)
N_G = RB // G_RB            # 25 groups

F32 = mybir.dt.float32
BF16 = mybir.dt.bfloat16
FP8 = mybir.dt.float8e4
ALU = mybir.AluOpType
ACTF = mybir.ActivationFunctionType
DR = mybir.MatmulPerfMode.DoubleRow


def _split_multiwait_instructions(nc):
    """The walrus build here rejects >1 sync-wait per instruction; hoist extra
    waits onto single-wait NOPs placed just before the instruction."""
    ctr = 0
    for fn in nc.m.functions:
        for bb in fn.blocks:
            insts = list(bb.instructions)
            if not any(
                i.sync_info is not None and len(i.sync_info.on_wait) > 1
                for i in insts
            ):
                continue
            new_insts = []
            for inst in insts:
                si = inst.sync_info
                if si is not None and len(si.on_wait) > 1:
                    waits = list(si.on_wait)
                    is_drain = type(inst).__name__ == "InstDrain"
                    keep = [] if is_drain else waits[-1:]
                    move = waits if is_drain else waits[:-1]
                    for w in move:
                        ctr += 1
                        new_insts.append(
                            mybir.InstNoOp(
                                name=f"I-wsplit-{ctr}",
                                engine=inst.engine,
                                sync_info=mybir.SyncInfo(on_wait=[w], on_update=[]),
                                text_hint="wsplit",
                            )
                        )
                    si.on_wait = keep
                new_insts.append(inst)
            bb.instructions = new_insts


def build_bass():
    nc = bass.Bass()
    # [p, group(25), rb(4), c(4), i(2), r(128)] flattened to [128, 102400]
    q_in = nc.declare_dram_parameter("q", [P, RB * D], FP8, isOutput=False)
    i_in = nc.declare_dram_parameter("im", [P, RB * D], FP8, isOutput=False)
    mask_in = nc.declare_dram_parameter("mask", [P, P], BF16, isOutput=False)
    out = nc.declare_dram_parameter("out", [P, SLOTS], F32, isOutput=True)

    qg = q_in[:].rearrange("p (g f) -> p g f", g=N_G)
    ig = i_in[:].rearrange("p (g f) -> p g f", g=N_G)

    with tile.TileContext(nc) as tc:
        with (
            tc.tile_pool(name="io", bufs=3) as io,
            tc.tile_pool(name="st", bufs=1) as st,
            tc.tile_pool(name="scx", bufs=2) as scx,
            tc.psum_pool(name="ps", bufs=6) as ps,
        ):
            mask = st.tile([P, P], BF16)
            nc.sync.dma_start(out=mask[:], in_=mask_in[:])

            dot_all = st.tile([P, RB], F32)
            nq2_all = st.tile([P, RB], F32)
            ni2_all = st.tile([P, RB], F32)

            for g in range(N_G):
                q_t = io.tile([P, G_RB * D], FP8, tag="q")
                i_t = io.tile([P, G_RB * D], FP8, tag="i")
                nc.sync.dma_start(out=q_t[:], in_=qg[:, g, :])
                nc.sync.dma_start(out=i_t[:], in_=ig[:, g, :])
                qv = q_t[:].rearrange("p (rb c i r) -> p rb c i r", rb=G_RB, c=4, i=2)
                iv = i_t[:].rearrange("p (rb c i r) -> p rb c i r", rb=G_RB, c=4, i=2)
                for rb in range(G_RB):
                    col = g * G_RB + rb
                    pt = ps.tile([P, 512], F32, tag="pt")
                    # group-major: DoubleRow accumulation groups must not
                    # interleave or the PE weight pipeline corrupts results.
                    for c in range(4):
                        nc.tensor.matmul(
                            pt[:, 0:128], qv[:, rb, c], qv[:, rb, c],
                            start=(c == 0), stop=(c == 3), perf_mode=DR)
                    for c in range(4):
                        nc.tensor.matmul(
                            pt[:, 128:256], qv[:, rb, c], iv[:, rb, c],
                            start=(c == 0), stop=(c == 3), perf_mode=DR)
                    for c in range(4):
                        nc.tensor.matmul(
                            pt[:, 256:384], iv[:, rb, c], iv[:, rb, c],
                            start=(c == 0), stop=(c == 3), perf_mode=DR)
                    scr = scx.tile([P, P], BF16, tag="scr")
                    nc.vector.scalar_tensor_tensor(
                        out=scr[:], in0=pt[:, 0:128], scalar=1.0, in1=mask[:],
                        op0=ALU.mult, op1=ALU.mult,
                        accum_out=nq2_all[:, col : col + 1])
                    nc.vector.scalar_tensor_tensor(
                        out=scr[:], in0=pt[:, 128:256], scalar=1.0, in1=mask[:],
                        op0=ALU.mult, op1=ALU.mult,
                        accum_out=dot_all[:, col : col + 1])
                    nc.vector.scalar_tensor_tensor(
                        out=scr[:], in0=pt[:, 256:384], scalar=1.0, in1=mask[:],
                        op0=ALU.mult, op1=ALU.mult,
                        accum_out=ni2_all[:, col : col + 1])

            # Tail: e = exp(dot * exp(-0.5*ln(max(nq2*ni2, eps^2)))), then the
            # two segmented reductions and the per-item fixup.
            prod = st.tile([P, RB], F32)
            nc.vector.tensor_tensor(
                out=prod[:], in0=nq2_all[:], in1=ni2_all[:], op=ALU.mult)
            nc.vector.tensor_scalar(
                out=prod[:], in0=prod[:], scalar1=EPS_COS * EPS_COS,
                scalar2=None, op0=ALU.max)
            nc.scalar.activation(out=prod[:], in_=prod[:], func=ACTF.Ln)
            nc.scalar.activation(out=prod[:], in_=prod[:], func=ACTF.Exp, scale=-0.5)
            cosv = st.tile([P, RB], F32)
            nc.vector.tensor_tensor(
                out=cosv[:], in0=dot_all[:], in1=prod[:], op=ALU.mult)
            e_all = st.tile([P, RB], F32)
            nc.scalar.activation(out=e_all[:], in_=cosv[:], func=ACTF.Exp)

            pos_sum = st.tile([P, SLOTS], F32)
            neg_sum = st.tile([P, SLOTS], F32)
            nc.vector.tensor_reduce(
                out=pos_sum[:],
                in_=e_all[:, :RB_POS].rearrange("p (s t) -> p s t", t=2),
                axis=mybir.AxisListType.X, op=ALU.add)
            nc.vector.tensor_reduce(
                out=neg_sum[:],
                in_=e_all[:, RB_POS:].rearrange("p (s t) -> p s t", t=23),
                axis=mybir.AxisListType.X, op=ALU.add)
            num = st.tile([P, SLOTS], F32)
            den = st.tile([P, SLOTS], F32)
            nc.vector.tensor_tensor(
                out=num[:], in0=neg_sum[:], in1=pos_sum[:], op=ALU.subtract)
            nc.vector.scalar_tensor_tensor(
                out=den[:], in0=pos_sum[:], scalar=EP, in1=neg_sum[:],
                op0=ALU.add, op1=ALU.add)
            rden = st.tile([P, SLOTS], F32)
            nc.vector.reciprocal(out=rden[:], in_=den[:])
            per_item = st.tile([P, SLOTS], F32)
            nc.vector.tensor_tensor(
                out=per_item[:], in0=num[:], in1=rden[:], op=ALU.mult)
            nc.sync.dma_start(out=out[:], in_=per_item[:])

    _split_multiwait_instructions(nc)
    return nc


_NC_CACHE = None


def _get_nc():
    global _NC_CACHE
    if _NC_CACHE is None:
        _NC_CACHE = build_bass()
    return _NC_CACHE


def _pack_core(shard_f8, n_rb, slots, rows_per_item):
    """[n_rb*128 rows, 1024] fp8 (core shard, original row order) ->
    [128, n_rb*1024] transposed-packed, DMA-grouped layout.

    Original row r_orig = rows_per_item*(4p + s) + k for item (p, s).
    Row-block j = rows_per_item*s + k gets column r = p:
      perm[j, r, d] = shard[rows_per_item*(4r + s) + k, d].
    Then d -> (c, i, p): tile[p_part, j, c, i, r] = perm[j, r, 256c+128i+p_part].
    """
    t = rows_per_item
    x = shard_f8.reshape(P, slots, t, D)            # [p, s, k, d]
    x = x.transpose(1, 2, 0, 3)                     # [s, k, r(=p), d]
    x = x.reshape(n_rb, P, 4, 2, P)                 # [j, r, c, i, p_part]
    x = x.transpose(4, 0, 2, 3, 1)                  # [p_part, j, c, i, r]
    return np.ascontiguousarray(x).reshape(P, n_rb * D)


def prepare_in_maps(question_embeddings_pos, question_embeddings_neg,
                    pos_image_embeddings, neg_image_embeddings):
    qp = np.asarray(question_embeddings_pos, dtype=np.float32).astype(
        ml_dtypes.float8_e4m3fn)
    qn = np.asarray(question_embeddings_neg, dtype=np.float32).astype(
        ml_dtypes.float8_e4m3fn)
    pi = np.asarray(pos_image_embeddings, dtype=np.float32).astype(
        ml_dtypes.float8_e4m3fn)
    ni = np.asarray(neg_image_embeddings, dtype=np.float32).astype(
        ml_dtypes.float8_e4m3fn)

    mask = np.eye(P, dtype=np.float32).astype(ml_dtypes.bfloat16)
    rp = 2 * ITEMS   # pos rows per core
    rn = 23 * ITEMS  # neg rows per core
    in_maps = []
    for c in range(N_CORES):
        qpos = _pack_core(qp[c * rp : (c + 1) * rp], RB_POS, SLOTS, 2)
        qneg = _pack_core(qn[c * rn : (c + 1) * rn], RB_NEG, SLOTS, 23)
        ipos = _pack_core(pi[c * rp : (c + 1) * rp], RB_POS, SLOTS, 2)
        ineg = _pack_core(ni[c * rn : (c + 1) * rn], RB_NEG, SLOTS, 23)
        in_maps.append({
            "q": np.concatenate([qpos, qneg], axis=1),
            "im": np.concatenate([ipos, ineg], axis=1),
            "mask": mask,
        })
    return in_maps


def kernel(question_embeddings_pos, question_embeddings_neg,
           pos_image_embeddings, neg_image_embeddings, batch_size=None,
           **_unused):
    in_maps = prepare_in_maps(
        question_embeddings_pos, question_embeddings_neg,
        pos_image_embeddings, neg_image_embeddings,
    )
    res = run_bass_kernel_spmd(_get_nc(), in_maps, list(range(N_CORES)))
    total = np.float64(0.0)
    for c in range(N_CORES):
        total += res.results[c]["out"].sum(dtype=np.float64)
    return np.float32(total)


# revision 3
# speedup vs baseline: 2.6461x; 1.3006x over previous
"""Trainium2 Bass kernel for the contrastive loss:

    epos = exp(cos_sim(q_pos, img_pos))   # [2B] rows, D=1024
    eneg = exp(cos_sim(q_neg, img_neg))   # [23B]
    pos_sum = segsum(epos, 2); neg_sum = segsum(eneg, 23)   # [B]
    loss = sum((neg_sum - pos_sum) / (pos_sum + neg_sum + 0.001))

Data-parallel over 8 NeuronCores: core c takes batch items [c*512, (c+1)*512).
Within a core the 12800 rows (1024 pos + 11776 neg) form 100 row-blocks of
128. All inputs are fp8-e4m3 (host cast): end-to-end fp8 error on this loss
is ~1e-6 relative (tolerance 2e-2), and fp8 halves HBM traffic to 25
MiB/core (~75 us at the measured ~350 GB/s/core).

Work splits across ALL THREE compute engines to beat the bf16 DVE/ACT
roofline (~190 us) and the all-PE roofline (~115 us):

- 76 row-blocks (TRANSPOSED layout tile[p,c,i,r] = X[r, 256c+128i+p]) run on
  the TensorEngine: per block and stat, 4 DoubleRow fp8 matmuls (K=256)
  accumulate a diag-bearing [128,128] PSUM block (~95 ns/MM measured); the
  DVE pulls each diagonal with one identity-mask scalar_tensor_tensor +
  accum (~250 ns). DoubleRow accumulation groups must not interleave (HW
  weight-pipeline hazard), so the 12 matmuls per block run group-major.
- 24 row-blocks (NATURAL layout, still fp8: DVE 1x / ACT are
  dtype-independent) run classically: row-dot via scalar_tensor_tensor with
  accum on DVE (~1.22 us), the two squared-norms via ACT Square with accum
  (~1.33 us each), keeping DVE and ACT saturated alongside the PE.

Row permutation on host: pos row-block j=2s+k holds, at column r, item
(r, s)'s k-th pos row; neg block j=8+23s+k likewise. Stats land at
stat[p=r, col=j], so the tail segsum is two tensor_reduce ops over
[p, (s,2)] / [p, (s,23)] followed by the per-item fixup, all identical to
the bf16 kernel. Host sums the [128,4] per-core outputs.
"""

import ml_dtypes
import numpy as np

import concourse.bass as bass
import concourse.tile as tile
from concourse import mybir
from concourse.bass_utils import run_bass_kernel_spmd

EPS_COS = 1e-8
EP = 0.001

N_CORES = 8
P = 128            # SBUF partitions
D = 1024           # embedding dim
B_FULL = 4096      # total batch items
ITEMS = B_FULL // N_CORES   # 512 items per core
SLOTS = ITEMS // P          # 4 items per partition
RB_POS = 8                  # pos row-blocks per core (1024 rows)
RB_NEG = 92                 # neg row-blocks per core (11776 rows)
RB = RB_POS + RB_NEG        # 100 row-blocks of 128 rows
RB_CLS = 24                 # classic-path row-blocks (last 24, all neg)
RB_PE = RB - RB_CLS         # 76 TensorEngine row-blocks

F32 = mybir.dt.float32
BF16 = mybir.dt.bfloat16
FP8 = mybir.dt.float8e4
ALU = mybir.AluOpType
ACTF = mybir.ActivationFunctionType
DR = mybir.MatmulPerfMode.DoubleRow


def _split_multiwait_instructions(nc):
    """The walrus build here rejects >1 sync-wait per instruction; hoist extra
    waits onto single-wait NOPs placed just before the instruction."""
    ctr = 0
    for fn in nc.m.functions:
        for bb in fn.blocks:
            insts = list(bb.instructions)
            if not any(
                i.sync_info is not None and len(i.sync_info.on_wait) > 1
                for i in insts
            ):
                continue
            new_insts = []
            for inst in insts:
                si = inst.sync_info
                if si is not None and len(si.on_wait) > 1:
                    waits = list(si.on_wait)
                    is_drain = type(inst).__name__ == "InstDrain"
                    keep = [] if is_drain else waits[-1:]
                    move = waits if is_drain else waits[:-1]
                    for w in move:
                        ctr += 1
                        new_insts.append(
                            mybir.InstNoOp(
                                name=f"I-wsplit-{ctr}",
                                engine=inst.engine,
                                sync_info=mybir.SyncInfo(on_wait=[w], on_update=[]),
                                text_hint="wsplit",
                            )
                        )
                    si.on_wait = keep
                new_insts.append(inst)
            bb.instructions = new_insts


# PE DMA groups: small first groups so the PE starts early.
PE_GROUPS = [1, 3] + [4] * ((RB_PE - 4) // 4)
assert sum(PE_GROUPS) == RB_PE
CLS_GROUPS = [4] * (RB_CLS // 4)


def build_bass():
    nc = bass.Bass()
    qt_in = nc.declare_dram_parameter("qt", [P, RB_PE * D], FP8, isOutput=False)
    it_in = nc.declare_dram_parameter("it", [P, RB_PE * D], FP8, isOutput=False)
    qc_in = nc.declare_dram_parameter("qc", [P, RB_CLS * D], FP8, isOutput=False)
    ic_in = nc.declare_dram_parameter("ic", [P, RB_CLS * D], FP8, isOutput=False)
    mask_in = nc.declare_dram_parameter("mask", [P, P], BF16, isOutput=False)
    out = nc.declare_dram_parameter("out", [P, SLOTS], F32, isOutput=True)

    qtv = qt_in[:].rearrange("p (rb f) -> p rb f", rb=RB_PE)
    itv = it_in[:].rearrange("p (rb f) -> p rb f", rb=RB_PE)
    qcv = qc_in[:].rearrange("p (rb f) -> p rb f", rb=RB_CLS)
    icv = ic_in[:].rearrange("p (rb f) -> p rb f", rb=RB_CLS)

    with tile.TileContext(nc) as tc:
        with (
            tc.tile_pool(name="io", bufs=3) as io,
            tc.tile_pool(name="ioc", bufs=2) as ioc,
            tc.tile_pool(name="st", bufs=1) as st,
            tc.tile_pool(name="scx", bufs=2) as scx,
            tc.psum_pool(name="ps", bufs=6) as ps,
        ):
            mask = st.tile([P, P], BF16)
            nc.sync.dma_start(out=mask[:], in_=mask_in[:])

            dot_all = st.tile([P, RB], F32)
            nq2_all = st.tile([P, RB], F32)
            ni2_all = st.tile([P, RB], F32)

            def pe_rb(qg, ig, rb, col):
                pt = ps.tile([P, 512], F32, tag="pt")
                # group-major: DoubleRow accumulation groups must not
                # interleave or the PE weight pipeline corrupts results.
                for c in range(4):
                    nc.tensor.matmul(
                        pt[:, 0:128], qg[:, rb, c], qg[:, rb, c],
                        start=(c == 0), stop=(c == 3), perf_mode=DR)
                for c in range(4):
                    nc.tensor.matmul(
                        pt[:, 128:256], qg[:, rb, c], ig[:, rb, c],
                        start=(c == 0), stop=(c == 3), perf_mode=DR)
                for c in range(4):
                    nc.tensor.matmul(
                        pt[:, 256:384], ig[:, rb, c], ig[:, rb, c],
                        start=(c == 0), stop=(c == 3), perf_mode=DR)
                scr = scx.tile([P, P], BF16, tag="scr")
                nc.vector.scalar_tensor_tensor(
                    out=scr[:], in0=pt[:, 0:128], scalar=1.0, in1=mask[:],
                    op0=ALU.mult, op1=ALU.mult,
                    accum_out=nq2_all[:, col : col + 1])
                nc.vector.scalar_tensor_tensor(
                    out=scr[:], in0=pt[:, 128:256], scalar=1.0, in1=mask[:],
                    op0=ALU.mult, op1=ALU.mult,
                    accum_out=dot_all[:, col : col + 1])
                nc.vector.scalar_tensor_tensor(
                    out=scr[:], in0=pt[:, 256:384], scalar=1.0, in1=mask[:],
                    op0=ALU.mult, op1=ALU.mult,
                    accum_out=ni2_all[:, col : col + 1])

            def cls_rb(qg, ig, rb, col):
                scr = scx.tile([P, D], BF16, tag="scrc")
                nc.vector.scalar_tensor_tensor(
                    out=scr[:], in0=qg[:, rb, :], scalar=1.0, in1=ig[:, rb, :],
                    op0=ALU.mult, op1=ALU.mult,
                    accum_out=dot_all[:, col : col + 1])
                nc.scalar.activation(
                    out=scr[:], in_=qg[:, rb, :], func=ACTF.Square,
                    accum_out=nq2_all[:, col : col + 1])
                nc.scalar.activation(
                    out=scr[:], in_=ig[:, rb, :], func=ACTF.Square,
                    accum_out=ni2_all[:, col : col + 1])

            # Interleave PE groups with classic groups: classic DMAs are
            # front-loaded so DVE/ACT have work for the whole kernel.
            cls_iter = iter(enumerate(CLS_GROUPS))
            rb0 = 0
            for gi, gsz in enumerate(PE_GROUPS):
                q_t = io.tile([P, 4 * D], FP8, tag="q")
                i_t = io.tile([P, 4 * D], FP8, tag="i")
                nc.sync.dma_start(out=q_t[:, : gsz * D], in_=qtv[:, rb0 : rb0 + gsz, :])
                nc.sync.dma_start(out=i_t[:, : gsz * D], in_=itv[:, rb0 : rb0 + gsz, :])
                qg = q_t[:].rearrange("p (rb c i r) -> p rb c i r", rb=4, c=4, i=2)
                ig = i_t[:].rearrange("p (rb c i r) -> p rb c i r", rb=4, c=4, i=2)
                for rb in range(gsz):
                    pe_rb(qg, ig, rb, rb0 + rb)
                rb0 += gsz
                # one classic group after every third PE group
                if gi % 3 == 0:
                    nxt = next(cls_iter, None)
                    if nxt is not None:
                        ci, csz = nxt
                        crb0 = ci * 4
                        q_c = ioc.tile([P, 4 * D], FP8, tag="qc")
                        i_c = ioc.tile([P, 4 * D], FP8, tag="ic")
                        nc.sync.dma_start(out=q_c[:], in_=qcv[:, crb0 : crb0 + 4, :])
                        nc.sync.dma_start(out=i_c[:], in_=icv[:, crb0 : crb0 + 4, :])
                        qcg = q_c[:].rearrange("p (rb f) -> p rb f", rb=4)
                        icg = i_c[:].rearrange("p (rb f) -> p rb f", rb=4)
                        for rb in range(csz):
                            cls_rb(qcg, icg, rb, RB_PE + crb0 + rb)
            for ci, csz in cls_iter:
                crb0 = ci * 4
                q_c = ioc.tile([P, 4 * D], FP8, tag="qc")
                i_c = ioc.tile([P, 4 * D], FP8, tag="ic")
                nc.sync.dma_start(out=q_c[:], in_=qcv[:, crb0 : crb0 + 4, :])
                nc.sync.dma_start(out=i_c[:], in_=icv[:, crb0 : crb0 + 4, :])
                qcg = q_c[:].rearrange("p (rb f) -> p rb f", rb=4)
                icg = i_c[:].rearrange("p (rb f) -> p rb f", rb=4)
                for rb in range(csz):
                    cls_rb(qcg, icg, rb, RB_PE + crb0 + rb)

            # Tail: e = exp(dot * exp(-0.5*ln(max(nq2*ni2, eps^2)))), then the
            # two segmented reductions and the per-item fixup.
            prod = st.tile([P, RB], F32)
            nc.vector.tensor_tensor(
                out=prod[:], in0=nq2_all[:], in1=ni2_all[:], op=ALU.mult)
            nc.vector.tensor_scalar(
                out=prod[:], in0=prod[:], scalar1=EPS_COS * EPS_COS,
                scalar2=None, op0=ALU.max)
            nc.scalar.activation(out=prod[:], in_=prod[:], func=ACTF.Ln)
            nc.scalar.activation(out=prod[:], in_=prod[:], func=ACTF.Exp, scale=-0.5)
            cosv = st.tile([P, RB], F32)
            nc.vector.tensor_tensor(
                out=cosv[:], in0=dot_all[:], in1=prod[:], op=ALU.mult)
            e_all = st.tile([P, RB], F32)
            nc.scalar.activation(out=e_all[:], in_=cosv[:], func=ACTF.Exp)

            pos_sum = st.tile([P, SLOTS], F32)
            neg_sum = st.tile([P, SLOTS], F32)
            nc.vector.tensor_reduce(
                out=pos_sum[:],
                in_=e_all[:, :RB_POS].rearrange("p (s t) -> p s t", t=2),
                axis=mybir.AxisListType.X, op=ALU.add)
            nc.vector.tensor_reduce(
                out=neg_sum[:],
                in_=e_all[:, RB_POS:].rearrange("p (s t) -> p s t", t=23),
                axis=mybir.AxisListType.X, op=ALU.add)
            num = st.tile([P, SLOTS], F32)
            den = st.tile([P, SLOTS], F32)
            nc.vector.tensor_tensor(
                out=num[:], in0=neg_sum[:], in1=pos_sum[:], op=ALU.subtract)
            nc.vector.scalar_tensor_tensor(
                out=den[:], in0=pos_sum[:], scalar=EP, in1=neg_sum[:],
                op0=ALU.add, op1=ALU.add)
            rden = st.tile([P, SLOTS], F32)
            nc.vector.reciprocal(out=rden[:], in_=den[:])
            per_item = st.tile([P, SLOTS], F32)
            nc.vector.tensor_tensor(
                out=per_item[:], in0=num[:], in1=rden[:], op=ALU.mult)
            nc.sync.dma_start(out=out[:], in_=per_item[:])

    _split_multiwait_instructions(nc)
    return nc


_NC_CACHE = None


def _get_nc():
    global _NC_CACHE
    if _NC_CACHE is None:
        _NC_CACHE = build_bass()
    return _NC_CACHE


def _permute_rows(shard_f8, n_rb, rows_per_item):
    """[n_rb*128, 1024] core shard -> perm[j, r, d] = row of item (r, s),
    k-th of its group, where j = rows_per_item*s + k."""
    t = rows_per_item
    x = shard_f8.reshape(P, SLOTS, t, D)            # [p, s, k, d]
    x = x.transpose(1, 2, 0, 3)                     # [s, k, r(=p), d]
    return x.reshape(n_rb, P, D)                    # [j, r, d]


def _pack_transposed(perm):
    """[n_rb, 128, 1024] -> [128, n_rb*1024]: tile[p, j, c, i, r] =
    perm[j, r, 256c + 128i + p]."""
    n_rb = perm.shape[0]
    x = perm.reshape(n_rb, P, 4, 2, P)              # [j, r, c, i, p]
    x = x.transpose(4, 0, 2, 3, 1)                  # [p, j, c, i, r]
    return np.ascontiguousarray(x).reshape(P, n_rb * D)


def _pack_natural(perm):
    """[n_rb, 128, 1024] -> [128, n_rb*1024]: tile[r, j, d] = perm[j, r, d]."""
    n_rb = perm.shape[0]
    x = perm.transpose(1, 0, 2)                     # [r, j, d]
    return np.ascontiguousarray(x).reshape(P, n_rb * D)


def prepare_in_maps(question_embeddings_pos, question_embeddings_neg,
                    pos_image_embeddings, neg_image_embeddings):
    qp = np.asarray(question_embeddings_pos, dtype=np.float32).astype(
        ml_dtypes.float8_e4m3fn)
    qn = np.asarray(question_embeddings_neg, dtype=np.float32).astype(
        ml_dtypes.float8_e4m3fn)
    pi = np.asarray(pos_image_embeddings, dtype=np.float32).astype(
        ml_dtypes.float8_e4m3fn)
    ni = np.asarray(neg_image_embeddings, dtype=np.float32).astype(
        ml_dtypes.float8_e4m3fn)

    mask = np.eye(P, dtype=np.float32).astype(ml_dtypes.bfloat16)
    rp = 2 * ITEMS   # pos rows per core
    rn = 23 * ITEMS  # neg rows per core
    n_split = RB_PE - RB_POS   # neg row-blocks on the PE path
    in_maps = []
    for c in range(N_CORES):
        pq = _permute_rows(qp[c * rp : (c + 1) * rp], RB_POS, 2)
        pnq = _permute_rows(qn[c * rn : (c + 1) * rn], RB_NEG, 23)
        pi_ = _permute_rows(pi[c * rp : (c + 1) * rp], RB_POS, 2)
        pni = _permute_rows(ni[c * rn : (c + 1) * rn], RB_NEG, 23)
        in_maps.append({
            "qt": np.concatenate(
                [_pack_transposed(pq), _pack_transposed(pnq[:n_split])], axis=1),
            "it": np.concatenate(
                [_pack_transposed(pi_), _pack_transposed(pni[:n_split])], axis=1),
            "qc": _pack_natural(pnq[n_split:]),
            "ic": _pack_natural(pni[n_split:]),
            "mask": mask,
        })
    return in_maps


def kernel(question_embeddings_pos, question_embeddings_neg,
           pos_image_embeddings, neg_image_embeddings, batch_size=None,
           **_unused):
    in_maps = prepare_in_maps(
        question_embeddings_pos, question_embeddings_neg,
        pos_image_embeddings, neg_image_embeddings,
    )
    res = run_bass_kernel_spmd(_get_nc(), in_maps, list(range(N_CORES)))
    total = np.float64(0.0)
    for c in range(N_CORES):
        total += res.results[c]["out"].sum(dtype=np.float64)
    return np.float32(total)


# revision 4
# speedup vs baseline: 2.7870x; 1.0533x over previous
"""Trainium2 Bass kernel for the contrastive loss:

    epos = exp(cos_sim(q_pos, img_pos))   # [2B] rows, D=1024
    eneg = exp(cos_sim(q_neg, img_neg))   # [23B]
    pos_sum = segsum(epos, 2); neg_sum = segsum(eneg, 23)   # [B]
    loss = sum((neg_sum - pos_sum) / (pos_sum + neg_sum + 0.001))

Data-parallel over 8 NeuronCores: core c takes batch items [c*512, (c+1)*512).
Within a core the 12800 rows (1024 pos + 11776 neg) form 100 row-blocks of
128. All inputs are fp8-e4m3 (host cast): end-to-end fp8 error on this loss
is ~1e-6 relative (tolerance 2e-2), and fp8 halves HBM traffic to 25
MiB/core (~75 us at the measured ~350 GB/s/core).

Work splits across ALL THREE compute engines to beat the bf16 DVE/ACT
roofline (~190 us) and the all-PE roofline (~115 us):

- 76 row-blocks (TRANSPOSED layout tile[p,c,i,r] = X[r, 256c+128i+p]) run on
  the TensorEngine: per block and stat, 4 DoubleRow fp8 matmuls (K=256)
  accumulate a diag-bearing [128,128] PSUM block (~95 ns/MM measured); the
  DVE pulls each diagonal with one identity-mask scalar_tensor_tensor +
  accum (~250 ns). DoubleRow accumulation groups must not interleave (HW
  weight-pipeline hazard), so the 12 matmuls per block run group-major.
- 24 row-blocks (NATURAL layout, still fp8: DVE 1x / ACT are
  dtype-independent) run classically: row-dot via scalar_tensor_tensor with
  accum on DVE (~1.22 us), the two squared-norms via ACT Square with accum
  (~1.33 us each), keeping DVE and ACT saturated alongside the PE.

Row permutation on host: pos row-block j=2s+k holds, at column r, item
(r, s)'s k-th pos row; neg block j=8+23s+k likewise. Stats land at
stat[p=r, col=j], so the tail segsum is two tensor_reduce ops over
[p, (s,2)] / [p, (s,23)] followed by the per-item fixup, all identical to
the bf16 kernel. Host sums the [128,4] per-core outputs.
"""

import ml_dtypes
import numpy as np

import concourse.bass as bass
import concourse.tile as tile
from concourse import mybir
from concourse.bass_utils import run_bass_kernel_spmd

EPS_COS = 1e-8
EP = 0.001

N_CORES = 8
P = 128            # SBUF partitions
D = 1024           # embedding dim
B_FULL = 4096      # total batch items
ITEMS = B_FULL // N_CORES   # 512 items per core
SLOTS = ITEMS // P          # 4 items per partition
RB_POS = 8                  # pos row-blocks per core (1024 rows)
RB_NEG = 92                 # neg row-blocks per core (11776 rows)
RB = RB_POS + RB_NEG        # 100 row-blocks of 128 rows
RB_CLS = 20                 # classic-path row-blocks (last 20, all neg)
RB_PE = RB - RB_CLS         # 76 TensorEngine row-blocks

F32 = mybir.dt.float32
BF16 = mybir.dt.bfloat16
FP8 = mybir.dt.float8e4
ALU = mybir.AluOpType
ACTF = mybir.ActivationFunctionType
DR = mybir.MatmulPerfMode.DoubleRow


def _split_multiwait_instructions(nc):
    """The walrus build here rejects >1 sync-wait per instruction; hoist extra
    waits onto single-wait NOPs placed just before the instruction."""
    ctr = 0
    for fn in nc.m.functions:
        for bb in fn.blocks:
            insts = list(bb.instructions)
            if not any(
                i.sync_info is not None and len(i.sync_info.on_wait) > 1
                for i in insts
            ):
                continue
            new_insts = []
            for inst in insts:
                si = inst.sync_info
                if si is not None and len(si.on_wait) > 1:
                    waits = list(si.on_wait)
                    is_drain = type(inst).__name__ == "InstDrain"
                    keep = [] if is_drain else waits[-1:]
                    move = waits if is_drain else waits[:-1]
                    for w in move:
                        ctr += 1
                        new_insts.append(
                            mybir.InstNoOp(
                                name=f"I-wsplit-{ctr}",
                                engine=inst.engine,
                                sync_info=mybir.SyncInfo(on_wait=[w], on_update=[]),
                                text_hint="wsplit",
                            )
                        )
                    si.on_wait = keep
                new_insts.append(inst)
            bb.instructions = new_insts


# PE DMA groups: small first groups so the PE starts early.
PE_GROUPS = [1, 2, 3, 2] + [4] * ((RB_PE - 8) // 4)
assert sum(PE_GROUPS) == RB_PE
# classic groups of 2 row-blocks: small DMA bursts that don't starve the PE
CLS_GROUPS = [2] * (RB_CLS // 2)


def build_bass():
    nc = bass.Bass()
    qt_in = nc.declare_dram_parameter("qt", [P, RB_PE * D], FP8, isOutput=False)
    it_in = nc.declare_dram_parameter("it", [P, RB_PE * D], FP8, isOutput=False)
    qc_in = nc.declare_dram_parameter("qc", [P, RB_CLS * D], FP8, isOutput=False)
    ic_in = nc.declare_dram_parameter("ic", [P, RB_CLS * D], FP8, isOutput=False)
    mask_in = nc.declare_dram_parameter("mask", [P, P], BF16, isOutput=False)
    out = nc.declare_dram_parameter("out", [P, SLOTS], F32, isOutput=True)

    qtv = qt_in[:].rearrange("p (rb f) -> p rb f", rb=RB_PE)
    itv = it_in[:].rearrange("p (rb f) -> p rb f", rb=RB_PE)
    qcv = qc_in[:].rearrange("p (rb f) -> p rb f", rb=RB_CLS)
    icv = ic_in[:].rearrange("p (rb f) -> p rb f", rb=RB_CLS)

    with tile.TileContext(nc) as tc:
        with (
            tc.tile_pool(name="io", bufs=5) as io,
            tc.tile_pool(name="ioc", bufs=3) as ioc,
            tc.tile_pool(name="st", bufs=1) as st,
            tc.tile_pool(name="scx", bufs=2) as scx,
            tc.psum_pool(name="ps", bufs=6) as ps,
        ):
            mask = st.tile([P, P], BF16)
            nc.sync.dma_start(out=mask[:], in_=mask_in[:])

            dot_all = st.tile([P, RB], F32)
            nq2_all = st.tile([P, RB], F32)
            ni2_all = st.tile([P, RB], F32)

            def pe_rb(qg, ig, rb, col):
                pt = ps.tile([P, 512], F32, tag="pt")
                # group-major: DoubleRow accumulation groups must not
                # interleave or the PE weight pipeline corrupts results.
                for c in range(4):
                    nc.tensor.matmul(
                        pt[:, 0:128], qg[:, rb, c], qg[:, rb, c],
                        start=(c == 0), stop=(c == 3), perf_mode=DR)
                for c in range(4):
                    nc.tensor.matmul(
                        pt[:, 128:256], qg[:, rb, c], ig[:, rb, c],
                        start=(c == 0), stop=(c == 3), perf_mode=DR)
                for c in range(4):
                    nc.tensor.matmul(
                        pt[:, 256:384], ig[:, rb, c], ig[:, rb, c],
                        start=(c == 0), stop=(c == 3), perf_mode=DR)
                scr = scx.tile([P, P], BF16, tag="scr")
                nc.vector.scalar_tensor_tensor(
                    out=scr[:], in0=pt[:, 0:128], scalar=1.0, in1=mask[:],
                    op0=ALU.mult, op1=ALU.mult,
                    accum_out=nq2_all[:, col : col + 1])
                nc.vector.scalar_tensor_tensor(
                    out=scr[:], in0=pt[:, 128:256], scalar=1.0, in1=mask[:],
                    op0=ALU.mult, op1=ALU.mult,
                    accum_out=dot_all[:, col : col + 1])
                nc.vector.scalar_tensor_tensor(
                    out=scr[:], in0=pt[:, 256:384], scalar=1.0, in1=mask[:],
                    op0=ALU.mult, op1=ALU.mult,
                    accum_out=ni2_all[:, col : col + 1])

            def cls_rb(qg, ig, rb, col):
                scr = scx.tile([P, D], BF16, tag="scrc")
                nc.vector.scalar_tensor_tensor(
                    out=scr[:], in0=qg[:, rb, :], scalar=1.0, in1=ig[:, rb, :],
                    op0=ALU.mult, op1=ALU.mult,
                    accum_out=dot_all[:, col : col + 1])
                nc.scalar.activation(
                    out=scr[:], in_=qg[:, rb, :], func=ACTF.Square,
                    accum_out=nq2_all[:, col : col + 1])
                nc.scalar.activation(
                    out=scr[:], in_=ig[:, rb, :], func=ACTF.Square,
                    accum_out=ni2_all[:, col : col + 1])

            # Interleave PE groups with classic groups: classic DMAs are
            # front-loaded so DVE/ACT have work for the whole kernel.
            cls_iter = iter(enumerate(CLS_GROUPS))
            rb0 = 0
            for gi, gsz in enumerate(PE_GROUPS):
                q_t = io.tile([P, 4 * D], FP8, tag="q")
                i_t = io.tile([P, 4 * D], FP8, tag="i")
                nc.sync.dma_start(out=q_t[:, : gsz * D], in_=qtv[:, rb0 : rb0 + gsz, :])
                nc.sync.dma_start(out=i_t[:, : gsz * D], in_=itv[:, rb0 : rb0 + gsz, :])
                qg = q_t[:].rearrange("p (rb c i r) -> p rb c i r", rb=4, c=4, i=2)
                ig = i_t[:].rearrange("p (rb c i r) -> p rb c i r", rb=4, c=4, i=2)
                for rb in range(gsz):
                    pe_rb(qg, ig, rb, rb0 + rb)
                rb0 += gsz
                # one 2-row-block classic group after every other PE group
                if gi % 2 == 0:
                    nxt = next(cls_iter, None)
                    if nxt is not None:
                        ci, csz = nxt
                        crb0 = ci * 2
                        q_c = ioc.tile([P, 2 * D], FP8, tag="qc")
                        i_c = ioc.tile([P, 2 * D], FP8, tag="ic")
                        nc.sync.dma_start(out=q_c[:], in_=qcv[:, crb0 : crb0 + 2, :])
                        nc.sync.dma_start(out=i_c[:], in_=icv[:, crb0 : crb0 + 2, :])
                        qcg = q_c[:].rearrange("p (rb f) -> p rb f", rb=2)
                        icg = i_c[:].rearrange("p (rb f) -> p rb f", rb=2)
                        for rb in range(csz):
                            cls_rb(qcg, icg, rb, RB_PE + crb0 + rb)
            for ci, csz in cls_iter:
                crb0 = ci * 2
                q_c = ioc.tile([P, 2 * D], FP8, tag="qc")
                i_c = ioc.tile([P, 2 * D], FP8, tag="ic")
                nc.sync.dma_start(out=q_c[:], in_=qcv[:, crb0 : crb0 + 2, :])
                nc.sync.dma_start(out=i_c[:], in_=icv[:, crb0 : crb0 + 2, :])
                qcg = q_c[:].rearrange("p (rb f) -> p rb f", rb=2)
                icg = i_c[:].rearrange("p (rb f) -> p rb f", rb=2)
                for rb in range(csz):
                    cls_rb(qcg, icg, rb, RB_PE + crb0 + rb)

            # Tail: e = exp(dot * exp(-0.5*ln(max(nq2*ni2, eps^2)))), then the
            # two segmented reductions and the per-item fixup.
            prod = st.tile([P, RB], F32)
            nc.vector.tensor_tensor(
                out=prod[:], in0=nq2_all[:], in1=ni2_all[:], op=ALU.mult)
            nc.vector.tensor_scalar(
                out=prod[:], in0=prod[:], scalar1=EPS_COS * EPS_COS,
                scalar2=None, op0=ALU.max)
            nc.scalar.activation(out=prod[:], in_=prod[:], func=ACTF.Ln)
            nc.scalar.activation(out=prod[:], in_=prod[:], func=ACTF.Exp, scale=-0.5)
            cosv = st.tile([P, RB], F32)
            nc.vector.tensor_tensor(
                out=cosv[:], in0=dot_all[:], in1=prod[:], op=ALU.mult)
            e_all = st.tile([P, RB], F32)
            nc.scalar.activation(out=e_all[:], in_=cosv[:], func=ACTF.Exp)

            pos_sum = st.tile([P, SLOTS], F32)
            neg_sum = st.tile([P, SLOTS], F32)
            nc.vector.tensor_reduce(
                out=pos_sum[:],
                in_=e_all[:, :RB_POS].rearrange("p (s t) -> p s t", t=2),
                axis=mybir.AxisListType.X, op=ALU.add)
            nc.vector.tensor_reduce(
                out=neg_sum[:],
                in_=e_all[:, RB_POS:].rearrange("p (s t) -> p s t", t=23),
                axis=mybir.AxisListType.X, op=ALU.add)
            num = st.tile([P, SLOTS], F32)
            den = st.tile([P, SLOTS], F32)
            nc.vector.tensor_tensor(
                out=num[:], in0=neg_sum[:], in1=pos_sum[:], op=ALU.subtract)
            nc.vector.scalar_tensor_tensor(
                out=den[:], in0=pos_sum[:], scalar=EP, in1=neg_sum[:],
                op0=ALU.add, op1=ALU.add)
            rden = st.tile([P, SLOTS], F32)
            nc.vector.reciprocal(out=rden[:], in_=den[:])
            per_item = st.tile([P, SLOTS], F32)
            nc.vector.tensor_tensor(
                out=per_item[:], in0=num[:], in1=rden[:], op=ALU.mult)
            nc.sync.dma_start(out=out[:], in_=per_item[:])

    _split_multiwait_instructions(nc)
    return nc


_NC_CACHE = None


def _get_nc():
    global _NC_CACHE
    if _NC_CACHE is None:
        _NC_CACHE = build_bass()
    return _NC_CACHE


def _permute_rows(shard_f8, n_rb, rows_per_item):
    """[n_rb*128, 1024] core shard -> perm[j, r, d] = row of item (r, s),
    k-th of its group, where j = rows_per_item*s + k."""
    t = rows_per_item
    x = shard_f8.reshape(P, SLOTS, t, D)            # [p, s, k, d]
    x = x.transpose(1, 2, 0, 3)                     # [s, k, r(=p), d]
    return x.reshape(n_rb, P, D)                    # [j, r, d]


def _pack_transposed(perm):
    """[n_rb, 128, 1024] -> [128, n_rb*1024]: tile[p, j, c, i, r] =
    perm[j, r, 256c + 128i + p]."""
    n_rb = perm.shape[0]
    x = perm.reshape(n_rb, P, 4, 2, P)              # [j, r, c, i, p]
    x = x.transpose(4, 0, 2, 3, 1)                  # [p, j, c, i, r]
    return np.ascontiguousarray(x).reshape(P, n_rb * D)


def _pack_natural(perm):
    """[n_rb, 128, 1024] -> [128, n_rb*1024]: tile[r, j, d] = perm[j, r, d]."""
    n_rb = perm.shape[0]
    x = perm.transpose(1, 0, 2)                     # [r, j, d]
    return np.ascontiguousarray(x).reshape(P, n_rb * D)


def prepare_in_maps(question_embeddings_pos, question_embeddings_neg,
                    pos_image_embeddings, neg_image_embeddings):
    qp = np.asarray(question_embeddings_pos, dtype=np.float32).astype(
        ml_dtypes.float8_e4m3fn)
    qn = np.asarray(question_embeddings_neg, dtype=np.float32).astype(
        ml_dtypes.float8_e4m3fn)
    pi = np.asarray(pos_image_embeddings, dtype=np.float32).astype(
        ml_dtypes.float8_e4m3fn)
    ni = np.asarray(neg_image_embeddings, dtype=np.float32).astype(
        ml_dtypes.float8_e4m3fn)

    mask = np.eye(P, dtype=np.float32).astype(ml_dtypes.bfloat16)
    rp = 2 * ITEMS   # pos rows per core
    rn = 23 * ITEMS  # neg rows per core
    n_split = RB_PE - RB_POS   # neg row-blocks on the PE path
    in_maps = []
    for c in range(N_CORES):
        pq = _permute_rows(qp[c * rp : (c + 1) * rp], RB_POS, 2)
        pnq = _permute_rows(qn[c * rn : (c + 1) * rn], RB_NEG, 23)
        pi_ = _permute_rows(pi[c * rp : (c + 1) * rp], RB_POS, 2)
        pni = _permute_rows(ni[c * rn : (c + 1) * rn], RB_NEG, 23)
        in_maps.append({
            "qt": np.concatenate(
                [_pack_transposed(pq), _pack_transposed(pnq[:n_split])], axis=1),
            "it": np.concatenate(
                [_pack_transposed(pi_), _pack_transposed(pni[:n_split])], axis=1),
            "qc": _pack_natural(pnq[n_split:]),
            "ic": _pack_natural(pni[n_split:]),
            "mask": mask,
        })
    return in_maps


def kernel(question_embeddings_pos, question_embeddings_neg,
           pos_image_embeddings, neg_image_embeddings, batch_size=None,
           **_unused):
    in_maps = prepare_in_maps(
        question_embeddings_pos, question_embeddings_neg,
        pos_image_embeddings, neg_image_embeddings,
    )
    res = run_bass_kernel_spmd(_get_nc(), in_maps, list(range(N_CORES)))
    total = np.float64(0.0)
    for c in range(N_CORES):
        total += res.results[c]["out"].sum(dtype=np.float64)
    return np.float32(total)
